# revision 1
# baseline (speedup 1.0000x reference)
"""Trainium2 Bass kernel for the segment_reduce loss (nn_Loss_65996467471179).

Strategy (data-parallel over curves):
  - C=65536 curves of L=256 points. Shard curves across 8 cores (8192 each).
  - Each core streams its 5 big arrays (An, A_r, Ac, Aj, Ap) once from HBM in
    [128, 2048] chunks (8 curves per partition), computes all per-curve and
    global partial reductions on-chip, and writes a small [128, 272] float32
    accumulator block back to DRAM.
  - Ci is only read at end-of-curve indices; that gather plus all C-length /
    O(4)-length pure-input terms (correlation moments, Rd25/dHa/Topt sign
    penalties) are folded on the host, which also combines the 8 cores'
    partial blocks into the final scalar in float64.

Per-curve math on device (curve rows live along the free axis, 8 per
partition):
  Acj   = Ac - Aj                      (GPSIMD)
  A     = |Acj| with fused per-curve accum sum|Acj|  (8 ACT slices/chunk)
  mn    = min_l A                      (DVE segmented 3D reduce)
  sAcj  = sum_l Acj                    (8 ACT Identity slices w/ accum_out)
  gint  = sum_l (A == mn) * (1.1*Aj - Ap)  == 1.1*Aj[argmin] - Ap[argmin]
          (fused DVE scalar_tensor_tensor: is_equal -> mult -> accum)
  ls_Ac = (sAbs+sAcj)/2, ls_Aj = (sAbs-sAcj)/2
  plus global sums of (An-A_r)^2 (GPSIMD sub + ACT Square accum) and
  relu(-Ap) (DVE tensor_scalar min-accum), and end-of-curve columns.

Engine balance per core (modeled): DVE ~74us, ACT ~82us, GPSIMD ~75us,
DMA 40MB at ~360-425GB/s ~ 94-112us -> memory-bound. Measured ~91us/exec
(quiet device; rises to ~160us under co-tenant HBM contention).
Relative error vs the f32 jax reference: 7.3e-08.
"""

import os
import sys

import numpy as np

sys.path.insert(0, "/opt/trn_rl_repo")

import concourse.bass as bass
import concourse.bacc as bacc
import concourse.tile as tile
from concourse import mybir
from concourse.bass_utils import run_bass_kernel_spmd
from contextlib import ExitStack

NCORES = 8
C = 65536
L = 256
N = C * L
S = C // NCORES          # curves per core
NSH = S * L              # elements per core per big array
P = 128                  # partitions
F = 2048                 # elements per partition per chunk
J = F // L               # curves per partition per chunk
M = NSH // (P * F)       # chunks per core (8)
NCOL = M * J             # per-curve accumulator columns (64)

KELVIN = 273.15
FIT_AP_CI = 500.0
TARGET_R = 0.7

f32 = mybir.dt.float32

# accumulator block column layout
MSE0 = 0            # [M]  per-chunk per-partition sum (An-A_r)^2
APN0 = MSE0 + M     # [M]  per-chunk per-partition sum relu(-Ap)
P30 = APN0 + M      # [NCOL] relu(3*gint) per curve
LS0 = P30 + NCOL    # [NCOL] w*(relu(8-ls_Aj)+relu(8-ls_Ac)) per curve
E10 = LS0 + NCOL    # [NCOL] relu(Ap_end-Aj_end)*fitw per curve
E20 = E10 + NCOL    # [NCOL] relu(Aj_end-Ac_end) per curve
ACCW = E20 + NCOL   # 272


VARIANT = dict(
    inp_bufs=2,      # stream-input pool buffering
    wrk_bufs=2,      # work-tile pool buffering
    d_on_pool=True,  # An-A_r subtract on GPSIMD (else DVE)
    epi_on_pool=True,   # epilogue tensor_tensor ops on GPSIMD (else DVE)
    sabs_on_act=True,   # compute A=|Acj| as 8 ACT slices with accum_out=sAbs
                        # (drops the DVE sAbs reduce)
    sacj_on_act=True,   # per-curve sum(Acj) via 8 ACT Identity slices w/ accum
    apn_on_dve=True,    # sum relu(-Ap) via DVE tensor_scalar instead of ACT
    mse_on_dve=False,   # sum d^2 via DVE tensor_tensor_reduce -- DO NOT ENABLE:
                        # TensorTensorReduce with in0==in1 fails at runtime on HW
    dma_acj_first=True,   # issue Ac/Aj stream DMAs before Ap/An/Ar
    chunked_epi=False,    # run the epilogue per chunk (cols slice) so it
                          # overlaps streaming instead of trailing the loop
    split_acc=False,      # mse/apn accumulate into own tiles (no ACT/DVE
                          # cross-engine serialization on accT)
)


def _build_kernel(reps=None, variant=None):
    """reps=None: normal single-pass kernel. reps=R: wrap the whole body in a
    runtime For_i loop executing it R times (for HW timing via slope)."""
    OP = mybir.AluOpType
    AF = mybir.ActivationFunctionType
    AX = mybir.AxisListType
    v = dict(VARIANT)
    if variant:
        v.update(variant)

    nc = bacc.Bacc("TRN2", target_bir_lowering=False, debug=False, num_devices=NCORES)
    big = {
        nm: nc.declare_dram_parameter(nm, [NSH], f32, isOutput=False)
        for nm in ("An", "Ar", "Ac", "Aj", "Ap")
    }
    wdev = nc.declare_dram_parameter("wdev", [P, NCOL], f32, isOutput=False)
    fitw = nc.declare_dram_parameter("fitw", [P, NCOL], f32, isOutput=False)
    acc = nc.declare_dram_parameter("acc", [P, ACCW], f32, isOutput=True)

    with ExitStack() as ctx:
        tc = ctx.enter_context(tile.TileContext(nc))
        inp = ctx.enter_context(tc.tile_pool(name="inp", bufs=v["inp_bufs"]))
        wrk = ctx.enter_context(tc.tile_pool(name="wrk", bufs=v["wrk_bufs"]))
        per = ctx.enter_context(tc.tile_pool(name="per", bufs=1))

        accT = per.tile([P, ACCW], f32, tag="accT")
        mnB = per.tile([P, NCOL], f32, tag="mnB")
        sAcj = per.tile([P, NCOL], f32, tag="sAcj")
        sAbs = per.tile([P, NCOL], f32, tag="sAbs")
        gint = per.tile([P, NCOL], f32, tag="gint")
        eAp = per.tile([P, NCOL], f32, tag="eAp")
        eAj = per.tile([P, NCOL], f32, tag="eAj")
        eAc = per.tile([P, NCOL], f32, tag="eAc")
        wT = per.tile([P, NCOL], f32, tag="wT")
        fT = per.tile([P, NCOL], f32, tag="fT")
        junkD = per.tile([P, L], f32, tag="junkD")
        junkA = per.tile([P, F], f32, tag="junkA")
        junkS = per.tile([P, L], f32, tag="junkS")
        junkV = per.tile([P, F], f32, tag="junkV")
        t1 = per.tile([P, NCOL], f32, tag="t1")
        t2 = per.tile([P, NCOL], f32, tag="t2")
        r1 = per.tile([P, NCOL], f32, tag="r1")
        r2 = per.tile([P, NCOL], f32, tag="r2")
        b8 = per.tile([P, 1], f32, tag="b8")
        mseB = per.tile([P, M], f32, tag="mseB")
        apnB = per.tile([P, M], f32, tag="apnB")
        nc.vector.memset(b8, 8.0)

        nc.sync.dma_start(out=wT, in_=wdev[:])
        nc.sync.dma_start(out=fT, in_=fitw[:])

        def body():
            _trace_body(nc, tc, big, acc, inp, wrk, accT, mnB, sAcj, sAbs, gint,
                        eAp, eAj, eAc, wT, fT, junkD, junkA, junkS, junkV, t1, t2, r1, r2, b8,
                        mseB, apnB, v)

        if reps is None:
            body()
        else:
            with tc.For_i(0, reps, 1):
                body()

    nc.compile()
    return nc


def _trace_body(nc, tc, big, acc, inp, wrk, accT, mnB, sAcj, sAbs, gint,
                eAp, eAj, eAc, wT, fT, junkD, junkA, junkS, junkV, t1, t2, r1, r2, b8,
                mseB, apnB, v):
    OP = mybir.AluOpType
    AF = mybir.ActivationFunctionType
    AX = mybir.AxisListType
    if True:
        for m in range(M):
            t = {}
            dma_order = ("Ac", "Aj", "Ap", "An", "Ar") if v["dma_acj_first"] \
                else ("An", "Ar", "Ac", "Aj", "Ap")
            for nm in dma_order:
                t[nm] = inp.tile([P, F], f32, tag=nm, name=f"in_{nm}_{m}")
                src = big[nm][:].rearrange("(m p f) -> m p f", m=M, p=P, f=F)[m]
                nc.sync.dma_start(out=t[nm], in_=src)

            cols = slice(m * J, (m + 1) * J)

            # --- GPSIMD: the two elementwise 2-input streams + end copies ---
            d = wrk.tile([P, F], f32, tag="d")
            d_eng = nc.gpsimd if v["d_on_pool"] else nc.vector
            d_eng.tensor_tensor(out=d, in0=t["An"], in1=t["Ar"], op=OP.subtract)
            G = wrk.tile([P, F], f32, tag="G")
            nc.vector.scalar_tensor_tensor(
                out=G, in0=t["Aj"], scalar=1.1, in1=t["Ap"],
                op0=OP.mult, op1=OP.subtract,
            )
            for nm, dst in (("Ap", eAp), ("Aj", eAj), ("Ac", eAc)):
                ends = t[nm].rearrange("p (j l) -> p j l", l=L)[:, :, L - 1 : L]
                nc.gpsimd.tensor_copy(out=dst[:, cols], in_=ends)

            # --- global accumulations: sum d^2 and sum relu(-Ap) ---
            mse_dst = mseB[:, m : m + 1] if v["split_acc"] \
                else accT[:, MSE0 + m : MSE0 + m + 1]
            if v["mse_on_dve"]:
                nc.vector.tensor_tensor_reduce(
                    out=junkV, in0=d, in1=d, scale=1.0, scalar=0.0,
                    op0=OP.mult, op1=OP.add, accum_out=mse_dst,
                )
            else:
                nc.scalar.activation(
                    out=junkA, in_=d, func=AF.Square, accum_out=mse_dst,
                )
            apn_dst = apnB[:, m : m + 1] if v["split_acc"] \
                else accT[:, APN0 + m : APN0 + m + 1]
            if v["apn_on_dve"]:
                # accum = sum(min(Ap, 0)) = -sum(relu(-Ap)); negated in epilogue.
                # (with accum_out, op1 is the reduction op)
                nc.vector.tensor_scalar(
                    out=junkV, in0=t["Ap"], scalar1=0.0, scalar2=None,
                    op0=OP.min, op1=OP.add, accum_out=apn_dst,
                )
            else:
                nc.scalar.activation(
                    out=junkA, in_=t["Ap"], func=AF.Relu, scale=-1.0,
                    accum_out=apn_dst,
                )
            Acj = wrk.tile([P, F], f32, tag="Acj")
            nc.gpsimd.tensor_tensor(out=Acj, in0=t["Ac"], in1=t["Aj"], op=OP.subtract)
            A = wrk.tile([P, F], f32, tag="A")
            if v["sabs_on_act"]:
                # slice-wise Abs with fused per-curve accumulation on ACT
                for j in range(J):
                    c = m * J + j
                    nc.scalar.activation(
                        out=A[:, j * L : (j + 1) * L],
                        in_=Acj[:, j * L : (j + 1) * L],
                        func=AF.Abs,
                        accum_out=sAbs[:, c : c + 1],
                    )
            else:
                nc.scalar.activation(out=A, in_=Acj, func=AF.Abs)

            # --- DVE: segmented per-curve reduces + argmin-select ---
            Acj3 = Acj.rearrange("p (j l) -> p j l", l=L)
            A3 = A.rearrange("p (j l) -> p j l", l=L)
            nc.vector.tensor_reduce(out=mnB[:, cols], in_=A3, axis=AX.X, op=OP.min)
            if v["sacj_on_act"]:
                for j in range(J):
                    c = m * J + j
                    nc.scalar.activation(
                        out=junkS,
                        in_=Acj[:, j * L : (j + 1) * L],
                        func=AF.Identity,
                        accum_out=sAcj[:, c : c + 1],
                    )
            else:
                nc.vector.tensor_reduce(out=sAcj[:, cols], in_=Acj3, axis=AX.X, op=OP.add)
            if not v["sabs_on_act"]:
                nc.vector.tensor_reduce(out=sAbs[:, cols], in_=A3, axis=AX.X, op=OP.add)
            for j in range(J):
                c = m * J + j
                nc.vector.scalar_tensor_tensor(
                    out=junkD,
                    in0=A[:, j * L : (j + 1) * L],
                    scalar=mnB[:, c : c + 1],
                    in1=G[:, j * L : (j + 1) * L],
                    op0=OP.is_equal,
                    op1=OP.mult,
                    accum_out=gint[:, c : c + 1],
                )

        # --- epilogue on [128, W] column blocks (whole or per chunk) ---
        def epilogue(lo, hi):
            W = hi - lo
            cs = slice(lo, hi)
            epi = nc.gpsimd if v["epi_on_pool"] else nc.vector
            # ls penalty: relu(8-ls_Aj)+relu(8-ls_Ac), ls_* = (sAbs -+ sAcj)/2
            epi.tensor_tensor(out=t1[:, :W], in0=sAbs[:, cs], in1=sAcj[:, cs], op=OP.add)
            nc.scalar.activation(out=r1[:, :W], in_=t1[:, :W], func=AF.Relu, scale=-0.5, bias=b8)
            epi.tensor_tensor(out=t2[:, :W], in0=sAbs[:, cs], in1=sAcj[:, cs], op=OP.subtract)
            nc.scalar.activation(out=r2[:, :W], in_=t2[:, :W], func=AF.Relu, scale=-0.5, bias=b8)
            epi.tensor_tensor(out=t1[:, :W], in0=r1[:, :W], in1=r2[:, :W], op=OP.add)
            epi.tensor_tensor(out=accT[:, LS0 + lo : LS0 + hi], in0=t1[:, :W],
                              in1=wT[:, cs], op=OP.mult)
            # crossover penalty: 3*relu(gint) == relu(3*gint)
            nc.scalar.activation(out=accT[:, P30 + lo : P30 + hi], in_=gint[:, cs],
                                 func=AF.Relu, scale=3.0)
            # end-of-curve penalties
            epi.tensor_tensor(out=t2[:, :W], in0=eAp[:, cs], in1=eAj[:, cs], op=OP.subtract)
            nc.scalar.activation(out=r1[:, :W], in_=t2[:, :W], func=AF.Relu)
            epi.tensor_tensor(out=accT[:, E10 + lo : E10 + hi], in0=r1[:, :W],
                              in1=fT[:, cs], op=OP.mult)
            epi.tensor_tensor(out=t2[:, :W], in0=eAj[:, cs], in1=eAc[:, cs], op=OP.subtract)
            nc.scalar.activation(out=accT[:, E20 + lo : E20 + hi], in_=t2[:, :W], func=AF.Relu)

        if v["chunked_epi"]:
            for m in range(M):
                epilogue(m * J, (m + 1) * J)
        else:
            epilogue(0, NCOL)
        if v["split_acc"]:
            nc.scalar.copy(out=accT[:, MSE0 : MSE0 + M], in_=mseB)
            if v["apn_on_dve"]:
                nc.vector.tensor_scalar_mul(
                    out=accT[:, APN0 : APN0 + M], in0=apnB, scalar1=-1.0)
            else:
                nc.scalar.copy(out=accT[:, APN0 : APN0 + M], in_=apnB)
        elif v["apn_on_dve"]:
            apn_blk = accT[:, APN0 : APN0 + M]
            nc.vector.tensor_scalar_mul(out=apn_blk, in0=apn_blk, scalar1=-1.0)

        nc.sync.dma_start(out=acc[:], in_=accT)


_NC_CACHE = {}
LAST_RESULTS = None


def _get_nc(reps=None, variant=None):
    key = (reps, tuple(sorted((variant or {}).items())))
    if key not in _NC_CACHE:
        _NC_CACHE[key] = _build_kernel(reps, variant)
    return _NC_CACHE[key]


def _curve_layout(x_per_curve: np.ndarray) -> np.ndarray:
    """Map a per-curve [S] array for one core into the device [P, NCOL] layout:
    dev[p, m*J + j] corresponds to curve m*(P*J) + p*J + j."""
    return np.ascontiguousarray(
        x_per_curve.reshape(M, P, J).transpose(1, 0, 2).reshape(P, NCOL)
    )


def prep_in_maps(An_o, Ac_o, Aj_o, Ap_o, A_r, Ci, mask_lightresp):
    w_full = (mask_lightresp == 0).astype(np.float32)        # [C]
    Ci_end = np.ascontiguousarray(Ci[L - 1 :: L])            # [C]
    fit_full = ((Ci_end > FIT_AP_CI).astype(np.float32) * w_full)  # [C]

    in_maps = []
    for k in range(NCORES):
        cur = slice(k * S, (k + 1) * S)
        el = slice(k * NSH, (k + 1) * NSH)
        in_maps.append({
            "An": np.ascontiguousarray(An_o[el]),
            "Ar": np.ascontiguousarray(A_r[el]),
            "Ac": np.ascontiguousarray(Ac_o[el]),
            "Aj": np.ascontiguousarray(Aj_o[el]),
            "Ap": np.ascontiguousarray(Ap_o[el]),
            "wdev": _curve_layout(w_full[cur]),
            "fitw": _curve_layout(fit_full[cur]),
        })
    return in_maps


def kernel(An_o, Ac_o, Aj_o, Ap_o, A_r, Ci, Vcmax25, Jmax25, Rd25,
           dHa_Vcmax, dHa_Jmax, dHa_TPU, Topt_Vcmax, Topt_Jmax, Topt_TPU,
           mask_lightresp):
    An_o, Ac_o, Aj_o, Ap_o, A_r, Ci = (
        np.asarray(x) for x in (An_o, Ac_o, Aj_o, Ap_o, A_r, Ci))
    (Vcmax25, Jmax25, Rd25, dHa_Vcmax, dHa_Jmax, dHa_TPU,
     Topt_Vcmax, Topt_Jmax, Topt_TPU, mask_lightresp) = (
        np.asarray(x) for x in (Vcmax25, Jmax25, Rd25, dHa_Vcmax, dHa_Jmax,
                                dHa_TPU, Topt_Vcmax, Topt_Jmax, Topt_TPU,
                                mask_lightresp))
    nc = _get_nc()
    in_maps = prep_in_maps(An_o, Ac_o, Aj_o, Ap_o, A_r, Ci, mask_lightresp)

    try:
        res = run_bass_kernel_spmd(
            nc, in_maps, core_ids=list(range(NCORES)),
            trace=bool(int(os.environ.get("KERNEL_TRACE", "0"))),
        )
    except ModuleNotFoundError:
        # tracing requested but the axon NTFF profiling hook isn't shipped in
        # this container — rerun with tracing disabled
        os.environ["BASS_NEVER_TRACE"] = "1"
        res = run_bass_kernel_spmd(nc, in_maps, core_ids=list(range(NCORES)))
    global LAST_RESULTS
    LAST_RESULTS = res
    blocks = [r["acc"].astype(np.float64) for r in res.results]

    mse = sum(b[:, MSE0 : MSE0 + M].sum() for b in blocks)
    apn = sum(b[:, APN0 : APN0 + M].sum() for b in blocks)
    p3 = sum(b[:, P30 : P30 + NCOL].sum() for b in blocks)
    ls = sum(b[:, LS0 : LS0 + NCOL].sum() for b in blocks)
    e1 = sum(b[:, E10 : E10 + NCOL].sum() for b in blocks)
    e2 = sum(b[:, E20 : E20 + NCOL].sum() for b in blocks)

    # host-side terms (tiny inputs only)
    w = (mask_lightresp == 0).astype(np.float64)
    x = Jmax25.astype(np.float64)
    y = Vcmax25.astype(np.float64)
    nw = w.sum()
    if nw > 0:
        my = (w * y).sum() / nw
        mx = (w * x).sum() / nw
        vy = (y - my) * w
        vx = (x - mx) * w
        denom = np.sqrt((vx * vx).sum()) * np.sqrt((vy * vy).sum())
        cost = (vx * vy).sum() / denom if denom != 0.0 else np.nan
    else:
        cost = np.nan
    if np.isnan(cost):
        cost = 0.0
    cost = min(cost, TARGET_R)

    relu = lambda v: np.maximum(v, 0.0)
    loss = mse * 10.0 / N
    loss += TARGET_R - cost
    loss += relu(-Rd25.astype(np.float64)).sum()
    loss += relu(-dHa_Vcmax.astype(np.float64)).sum() * 10.0
    loss += relu(-dHa_Jmax.astype(np.float64)).sum()
    loss += relu(-dHa_TPU.astype(np.float64)).sum()
    loss += relu(KELVIN - Topt_Vcmax.astype(np.float64)).sum()
    loss += relu(KELVIN - Topt_Jmax.astype(np.float64)).sum()
    loss += relu(KELVIN - Topt_TPU.astype(np.float64)).sum()
    loss += apn
    loss += e1 * 0.15
    loss += e2
    loss += p3
    loss += ls

    return np.asarray(loss, dtype=np.float32)

